# revision 2
# baseline (speedup 1.0000x reference)
"""CBAM kernel for Trainium2 (Bass/Tile), data-parallel over batch on 8 NeuronCores.

Reference computation (per sample):
  ch_att = sigmoid(MLP(mean_hw(x)) + MLP(max_hw(x)))          # [C]
  y      = x * ch_att[:, None, None]
  sp     = conv7x7(concat([mean_c(y), max_c(y)]))             # [H, W]
  out    = y * sigmoid(sp)[None]

v3 design (per core, 2 samples of x[256,128,128] fp32):
  layout: [128 partitions (channel%128), 2 blocks, HW] tiles of 1024 px.
  - load fp32 tile; ACT copy->fp16 with accum_out gives channel sums and
    the fp16 working copy in ONE pass; fp32 buffer freed immediately.
  - channel max: running fp16 tensor_tensor(max) chain (TensorReduce has
    no DVE fast mode; TT fp16 runs 2x) + one small final reduce.
  - spatial stats: y0/y1 = x16*ch (tensor_scalar fp16, kept for the
    final), m2 = TT max into scratch; 128-partition max on gpsimd
    partition_all_reduce; channel-sum via PE matmuls (lhsT=ch16).
  - conv 7x7: banded matmuls SPLIT 3 WAYS on the h axis (contraction
    bases 0/32/64) so sp chunks become available while phase B is still
    running -> final/store pipeline right behind phase B, x16/y bufs
    recycle early, loads of the next sample stream with no bubble.
  - final: PE replicates sigmoid rows (ones x spf) -> PSUM fp32, ACT
    copy -> fp16 pr, DVE TT y*pr -> fp16 out staging, fp16 DMA store
    (host converts back to fp32).
"""

import numpy as np

B = 16          # full batch
N_CORES = 8
B_LOC = B // N_CORES   # 2 samples per core
C = 256
H = W = 128
HW = H * W      # 16384
R = 16
K = 7
NB = 2          # channel blocks of 128
FT = 1024       # px per tile
NT = HW // FT   # 16 tiles per sample
QS = 512        # PSUM fp32 matmul-group chunk (bank limit)
RPT = FT // W   # A0/A1 rows per tile (8)

# conv split: 4 strips of 32 output rows; each strip's contraction over h
# is covered by legal PE partition windows (base 0/32/64/96, span<=32, or
# base 0/64 span<=64). Low-side over-coverage is free (band zeros, deps on
# earlier A chunks); the tail window is tight so strips stream out early.
CONV_STRIPS = [
    (0, 32, [(0, 32), (32, 3)]),
    (32, 64, [(0, 64), (64, 3)]),
    (64, 96, [(32, 32), (64, 35)]),
    (96, 128, [(64, 64)]),
]

# 1-in-N of the per-sample sp-replications go to gpsimd partition_broadcast
# (rest via PE matmul + ACT psum copy). Measured: partition_broadcast is
# much slower than the cost model claims -> disabled.
PR_GPSIMD_EVERY = 10 ** 9

_CACHE = {}


def _build_program(repeat=1):
    import concourse.bass as bass
    import concourse.bacc as bacc
    import concourse.tile as tile
    from concourse import mybir, bass_isa, library_config
    from contextlib import ExitStack

    f32 = mybir.dt.float32
    f16 = mybir.dt.float16
    AF = mybir.ActivationFunctionType
    ALU = mybir.AluOpType
    AX = mybir.AxisListType

    nc = bacc.Bacc("TRN2", target_bir_lowering=False, debug=False)

    x_ext = nc.declare_dram_parameter("x", [B_LOC, C, H, W], f32, isOutput=False)
    w1t_ext = nc.declare_dram_parameter("w1t", [128, NB, R], f32, isOutput=False)
    w2t_ext = nc.declare_dram_parameter("w2t", [R, C], f32, isOutput=False)
    wb_ext = nc.declare_dram_parameter("wb", [128, 2 * K, 128], f16, isOutput=False)
    ones_ext = nc.declare_dram_parameter("ones1", [65, 128], f16, isOutput=False)
    onesc_ext = nc.declare_dram_parameter("onesc", [128, 1], f16, isOutput=False)
    out_ext = nc.declare_dram_parameter("out", [B_LOC, C, H, W], f16, isOutput=True)

    xr = x_ext[:, :, :, :].rearrange("b (g c) h w -> b c g (h w)", g=NB)
    outr = out_ext[:, :, :, :].rearrange("b (g c) h w -> b c g (h w)", g=NB)

    with tile.TileContext(nc) as tc, ExitStack() as ctx:
        consts = ctx.enter_context(tc.tile_pool(name="consts", bufs=1))
        x32p = ctx.enter_context(tc.tile_pool(name="x32", bufs=4))
        x16p = ctx.enter_context(tc.tile_pool(name="x16", bufs=18))
        outp = ctx.enter_context(tc.tile_pool(name="outp", bufs=4))
        mxp = ctx.enter_context(tc.tile_pool(name="mxp", bufs=2))
        m2p = ctx.enter_context(tc.tile_pool(name="m2p", bufs=2))
        spool = ctx.enter_context(tc.tile_pool(name="spool", bufs=2))
        spfp = ctx.enter_context(tc.tile_pool(name="spfp", bufs=1))
        prp = ctx.enter_context(tc.tile_pool(name="prp", bufs=2))
        smalls = ctx.enter_context(tc.tile_pool(name="smalls", bufs=2))
        pmlp = ctx.enter_context(
            tc.tile_pool(name="pmlp", bufs=1, space=bass.MemorySpace.PSUM))
        psp = ctx.enter_context(
            tc.tile_pool(name="psp", bufs=2, space=bass.MemorySpace.PSUM))
        ppq = ctx.enter_context(
            tc.tile_pool(name="ppq", bufs=3, space=bass.MemorySpace.PSUM))
        pconv = ctx.enter_context(
            tc.tile_pool(name="pconv", bufs=2, space=bass.MemorySpace.PSUM))

        nc.gpsimd.load_library(library_config.mlp)
        w1t = consts.tile([128, NB, R], f32, tag="w1t")
        nc.sync.dma_start(w1t[:, :, :], w1t_ext[:, :, :])
        w2t = consts.tile([R, C], f32, tag="w2t")
        nc.sync.dma_start(w2t[:, :], w2t_ext[:, :])
        wb = consts.tile([128, 2 * K, 128], f16, tag="wb")
        nc.sync.dma_start(wb[:, :, :], wb_ext[:, :, :])
        ones1 = consts.tile([65, 128], f16, tag="ones1")
        nc.sync.dma_start(ones1[:, :], ones_ext[:, :])
        onesc = consts.tile([128, 1], f16, tag="onesc")
        nc.sync.dma_start(onesc[:, :], onesc_ext[:, :])

        for s in [s_ for _ in range(repeat) for s_ in range(B_LOC)]:
            # ---------------- phase A: load, cvt->fp16, stats ------------
            xts16 = []
            sums = smalls.tile([128, NB, NT], f32, tag="sums")
            mAcc = mxp.tile([128, NB, FT], f16, tag="mAcc")
            for t in range(NT):
                xt = x32p.tile([128, NB, FT], f32, tag="xt")
                nc.sync.dma_start(xt[:, :, :], xr[s, :, :, t * FT:(t + 1) * FT])
                x6 = x16p.tile([128, NB, FT], f16, tag="x16")
                xts16.append(x6)
                for b in range(NB):
                    nc.scalar.activation(
                        x6[:, b, :], xt[:, b, :], AF.Copy,
                        accum_out=sums[:, b, t:t + 1])
                if t == 0:
                    nc.vector.tensor_copy(mAcc[:, :, :], x6[:, :, :])
                else:
                    nc.vector.tensor_tensor(
                        mAcc[:, :, :], mAcc[:, :, :], x6[:, :, :], op=ALU.max)

            # ---------------- MLP -> ch_att ------------------------------
            stats = smalls.tile([128, NB, 2], f32, tag="stats")
            for b in range(NB):
                nc.vector.tensor_reduce(
                    stats[:, b, 0:1], sums[:, b, :], axis=AX.X, op=ALU.add)
                nc.vector.tensor_scalar_mul(
                    stats[:, b, 0:1], stats[:, b, 0:1], 1.0 / HW)
                nc.vector.tensor_reduce(
                    stats[:, b, 1:2], mAcc[:, b, :], axis=AX.X, op=ALU.max)
            ph = pmlp.tile([R, 2], f32, tag="mlp", name="ph")
            nc.tensor.matmul(ph[:, :], w1t[:, 0, :], stats[:, 0, :],
                             start=True, stop=False)
            nc.tensor.matmul(ph[:, :], w1t[:, 1, :], stats[:, 1, :],
                             start=False, stop=True)
            hmlp = smalls.tile([R, 2], f32, tag="hmlp")
            nc.scalar.activation(hmlp[:, :], ph[:, :], AF.Relu)
            ch = smalls.tile([128, NB], f32, tag="ch")
            for b in range(NB):
                p2 = pmlp.tile([128, 2], f32, tag="mlp", name="p2")
                nc.tensor.matmul(p2[:, :], w2t[:, b * 128:(b + 1) * 128],
                                 hmlp[:, :], start=True, stop=True)
                tsum = smalls.tile([128, 1], f32, tag="tsum")
                nc.vector.tensor_reduce(
                    tsum[:, :], p2[:, :], axis=AX.X, op=ALU.add)
                nc.scalar.activation(ch[:, b:b + 1], tsum[:, :], AF.Sigmoid)
            # ---------------- phase B: scale x16 in place, spatial stats -
            # x6 <- x6 * ch (fp16, in place): the x16 buffers double as the
            # y tiles consumed by the final multiply.
            A0 = spool.tile([128, 128], f16, tag="A0")
            A1 = spool.tile([128, 128], f16, tag="A1")
            for u in range(NT):
                x6 = xts16[u]
                nc.vector.tensor_scalar_mul(x6[:, 0, :], x6[:, 0, :],
                                            ch[:, 0:1])
                nc.vector.tensor_scalar_mul(x6[:, 1, :], x6[:, 1, :],
                                            ch[:, 1:2])
                m2 = m2p.tile([128, FT], f16, tag="m2")
                nc.vector.tensor_tensor(m2[:, :], x6[:, 0, :], x6[:, 1, :],
                                        op=ALU.max)
                art = m2p.tile([128, FT], f16, tag="art")
                nc.gpsimd.partition_all_reduce(
                    art[:, :], m2[:, :], channels=128,
                    reduce_op=bass_isa.ReduceOp.max)
                nc.sync.dma_start(A1[RPT * u:RPT * (u + 1), :], art[0:1, :])
                sse = spool.tile([1, FT], f16, tag="sse")
                for q in range(FT // QS):
                    psA = psp.tile([1, QS], f32, tag="psA")
                    sl = slice(q * QS, (q + 1) * QS)
                    nc.tensor.matmul(psA[:, :], onesc[:, 0:1], x6[:, 0, sl],
                                     start=True, stop=False)
                    nc.tensor.matmul(psA[:, :], onesc[:, 0:1], x6[:, 1, sl],
                                     start=False, stop=True)
                    nc.scalar.activation(sse[:, sl], psA[:, :], AF.Copy)
                nc.sync.dma_start(A0[RPT * u:RPT * (u + 1), :], sse[:, :])

            # ------- conv 7x7, 4 strips so sp streams out early ----------
            # sp pixels land on partition 0 (two tiles) so both the PE
            # replication path and gpsimd partition_broadcast can read them
            spfA = spfp.tile([1, 8192], f16, tag="spfA")
            spfB = spfp.tile([1, 8192], f16, tag="spfB")
            taps = [(1, 3)] + [(c, dx) for c in (1, 0) for dx in range(K)
                               if not (c == 1 and dx == 3)]
            for v, (mlo, mhi, parts) in enumerate(CONV_STRIPS):
                pc = pconv.tile([32, 128], f32, tag="pc")
                n_mm = len(parts) * len(taps)
                i = 0
                for kb, kn in parts:
                    for c, dx in taps:
                        sh = dx - 3
                        dlo, dhi = max(0, -sh), 128 - max(0, sh)
                        A = A0 if c == 0 else A1
                        nc.tensor.matmul(
                            pc[:, dlo:dhi],
                            wb[kb:kb + kn, c * K + dx, mlo:mhi],
                            A[kb:kb + kn, dlo + sh:dhi + sh],
                            start=(i == 0), stop=(i == n_mm - 1))
                        i += 1
                spa = spool.tile([32, 128], f16, tag="spa")
                nc.scalar.activation(spa[:, :], pc[:, :], AF.Sigmoid)
                spfX = spfA if v < 2 else spfB
                nc.sync.dma_start(
                    spfX[0:1, 4096 * (v % 2):4096 * (v % 2) + 4096],
                    spa[:, :])

            # ---------------- final: out = y * sp replicas ---------------
            for t in range(NT):
                os_ = outp.tile([128, NB, FT], f16, tag="os")
                pr = prp.tile([128, FT], f16, tag="pr")
                rr, off = t // 8, (t % 8) * FT
                spfX = spfA if rr == 0 else spfB
                if t % PR_GPSIMD_EVERY == 0:
                    nc.gpsimd.partition_broadcast(
                        pr[:, :], spfX[0:1, off:off + FT], channels=128)
                else:
                    for q in range(FT // QS):
                        sl = slice(off + q * QS, off + (q + 1) * QS)
                        pq = ppq.tile([128, QS], f32, tag="pq")
                        nc.tensor.matmul(
                            pq[:, :], ones1[0:1, :], spfX[0:1, sl],
                            start=True, stop=True)
                        nc.scalar.activation(pr[:, q * QS:(q + 1) * QS],
                                             pq[:, :], AF.Copy)
                x6 = xts16[t]
                for b in range(NB):
                    nc.vector.tensor_tensor(
                        os_[:, b, :], x6[:, b, :], pr[:, :], op=ALU.mult)
                nc.sync.dma_start(outr[s, :, :, t * FT:(t + 1) * FT],
                                  os_[:, :, :])

    nc.compile()
    return nc


def get_program(repeat=1):
    key = ("nc", repeat)
    if key not in _CACHE:
        _CACHE[key] = _build_program(repeat)
    return _CACHE[key]


def _host_prep(w1, w2, wconv):
    w1 = np.asarray(w1, dtype=np.float32)
    w2 = np.asarray(w2, dtype=np.float32)
    wconv = np.asarray(wconv, dtype=np.float32)
    # w1t[p, b, j] = w1[j, b*128 + p]
    w1t = np.ascontiguousarray(w1.T.reshape(NB, 128, R).transpose(1, 0, 2))
    w2t = np.ascontiguousarray(w2.T)  # [R, C]
    # banded conv matrices: wb[h, c*K+dx, hp] = keff[c, h-hp+3, dx]
    # (dim0 = h = contraction/partition, dim2 = h' = output row; matches
    # the baseline-proven orientation: lhsT[k, m] = keff[k - m + 3])
    keff = wconv[0].copy()          # [2, K, K] (dy, dx)
    keff[0] /= C                    # fold the channel-mean divide
    hp = np.arange(128)[:, None]    # h  (contraction, partition dim)
    hh = np.arange(128)[None, :]    # h' (output row, free dim)
    dy = hp - hh + 3                # [128, 128]
    valid = (dy >= 0) & (dy < K)
    dyc = np.clip(dy, 0, K - 1)
    wb = np.zeros((128, 2 * K, 128), dtype=np.float32)
    for c in range(2):
        for dx in range(K):
            wb[:, c * K + dx, :] = np.where(valid, keff[c][dyc, dx], 0.0)
    ones1 = np.zeros((65, 128), dtype=np.float16)
    ones1[[0, 32], :] = 1.0
    onesc = np.ones((128, 1), dtype=np.float16)
    return w1t, w2t, wb.astype(np.float16), ones1, onesc


def _in_maps(x, w1, w2, wconv):
    x = np.ascontiguousarray(np.asarray(x, dtype=np.float32))
    assert x.shape == (B, C, H, W), x.shape
    w1t, w2t, wb, ones1, onesc = _host_prep(w1, w2, wconv)
    return [{
        "x": x[i * B_LOC:(i + 1) * B_LOC],
        "w1t": w1t, "w2t": w2t, "wb": wb, "ones1": ones1,
        "onesc": onesc,
    } for i in range(N_CORES)]


def kernel(x, w1, w2, wconv):
    from concourse.bass_utils import run_bass_kernel_spmd

    in_maps = _in_maps(x, w1, w2, wconv)
    nc = get_program()
    res = run_bass_kernel_spmd(nc, in_maps, list(range(N_CORES)))
    out = np.concatenate([res.results[i]["out"] for i in range(N_CORES)], axis=0)
    return out.astype(np.float32)

